# revision 10
# baseline (speedup 1.0000x reference)
"""GPU-preprocessor kernel for Trainium2 (Bass/Tile), 8-core data parallel.

Pipeline per image (NHWC f32 [1280, 960, 3] -> NCHW f32 [3, 640, 640]):
  1. bilinear resize 1280x960 -> 640x640, half-pixel centers, no antialias
     - H: exact 2x downscale -> out_row i = 0.5*(row 2i + row 2i+1)
     - W: 1.5x downscale, period 3 px -> 2 px:
         out j=2k   = 0.75*px[3k]   + 0.25*px[3k+1]
         out j=2k+1 = 0.25*px[3k+1] + 0.75*px[3k+2]
  2. x/255, (x-mean)/std folded into one affine per channel applied last.

Per 128-row tile (v = e + o is the vertical pair-sum, so out px pre-affine
is 0.125*(3*v_near + v_mid)):
  - DMA (SWDGE via gpsimd): one contiguous load [128, 5760] (row pairs)
  - GPSIMD+DVE: v = e + o (2880, column-split across both engines)
  - DVE: scalar_tensor_tensor t[even px] = (v_l * 3 + v_m) and
    t[odd px] = (v_r * 3 + v_m), (j c)-interleaved writes (3-float runs)
    (TensorScalarPtr is DVE-only on core v3; GPSIMD rejects it)
  - ACT: per channel, out_c = t_c * (0.125*s_c) + b_c with
    s_c = 1/(255*std_c), b_c = -mean_c/std_c; deinterleaves to planar
  - DMA (HWDGE via sync): store each [128, 640] channel plane as soon
    as its ACT finishes (evens out store traffic vs one big store)
"""

import numpy as np
from contextlib import ExitStack

import concourse.mybir as mybir
from concourse import bass
from concourse import tile
from concourse.bass_utils import run_bass_kernel_spmd

F32 = mybir.dt.float32

N_CORES = 8
B_FULL = 16
H_IN, W_IN, C = 1280, 960, 3
H_OUT, W_OUT = 640, 640
PER_B = B_FULL // N_CORES          # 2 images per core
TILE_P = 128                       # output rows per tile
N_TILES = H_OUT // TILE_P          # 5 tiles per image
FREE_IN = W_IN * C                 # 2880 floats per input row
FREE_PAIR = 2 * FREE_IN            # 5760 floats per row-pair
FREE_OUT = W_OUT * C               # 1920 floats per output row
V_SPLIT = 1024                     # GPSIMD's share of the vertical add
                                   # (64B-aligned so DVE's tail slice stays aligned)

_BUILT_CACHE = {}


def _build_nc():
    nc = bass.Bass()
    img = nc.declare_dram_parameter("images", [PER_B, H_IN, W_IN, C], F32, isOutput=False)
    sb = nc.declare_dram_parameter("sb", [TILE_P, 8], F32, isOutput=False)
    out = nc.declare_dram_parameter("out", [PER_B, C, H_OUT, W_OUT], F32, isOutput=True)

    with tile.TileContext(nc) as tc, ExitStack() as ctx:
        const_pool = ctx.enter_context(tc.tile_pool(name="const", bufs=1))
        in_pool = ctx.enter_context(tc.tile_pool(name="inp", bufs=4))
        t_pool = ctx.enter_context(tc.tile_pool(name="t", bufs=3))
        o_pool = ctx.enter_context(tc.tile_pool(name="o", bufs=3))

        sbt_raw = const_pool.tile([TILE_P, 8], F32, tag="sbt_raw")
        nc.sync.dma_start(sbt_raw[:], sb[:])
        # DVE-owned copy so downstream ACT ops don't need a DMA wait
        sbt = const_pool.tile([TILE_P, 8], F32, tag="sbt")
        nc.vector.tensor_copy(sbt[:], sbt_raw[:])

        for b in range(PER_B):
            # [640 row-pairs, 5760 floats] contiguous per pair
            src_pairs = img[b].rearrange("(pair two) w c -> pair (two w c)", two=2)
            for ti in range(N_TILES):
                i0 = ti * TILE_P

                tin = in_pool.tile([TILE_P, FREE_PAIR], F32, tag="tin")
                # SWDGE loads: keeps load issues off the SP ring, where store
                # waits (on ACT output) would head-of-line-block them.
                nc.gpsimd.dma_start(tin[:], src_pairs[i0:i0 + TILE_P, :])

                e = tin[:, 0:FREE_IN]
                o = tin[:, FREE_IN:FREE_PAIR]
                # vertical add in-place into the e-half (engines stream
                # element reads ahead of writes, same-index safe), split
                # DVE/GPSIMD to balance engine load
                v = e
                nc.gpsimd.tensor_add(v[:, 0:V_SPLIT], e[:, 0:V_SPLIT], o[:, 0:V_SPLIT])
                nc.vector.tensor_add(v[:, V_SPLIT:], e[:, V_SPLIT:], o[:, V_SPLIT:])

                # windows of v: [p, k, 9]; px 3k/3k+1/3k+2 are floats
                # 0:3 / 3:6 / 6:9 of each 9-group
                v9 = v.rearrange("p (k nine) -> p k nine", nine=9)
                v_l = v9[:, :, 0:3]
                v_m = v9[:, :, 3:6]
                v_r = v9[:, :, 6:9]

                # pre-affine output, (j c)-interleaved so stt writes land in
                # contiguous 3-float runs: even px floats 6k+c, odd 6k+3+c
                t = t_pool.tile([TILE_P, FREE_OUT], F32, tag="t")
                t6 = t[:].rearrange("p (k six) -> p k six", six=6)
                nc.vector.scalar_tensor_tensor(
                    t6[:, :, 0:3], v_l, 3.0, v_m,
                    mybir.AluOpType.mult, mybir.AluOpType.add)
                nc.vector.scalar_tensor_tensor(
                    t6[:, :, 3:6], v_r, 3.0, v_m,
                    mybir.AluOpType.mult, mybir.AluOpType.add)

                # per-channel affine (scale = 0.125*s_c) deinterleaves
                # (stride-3 reads, planar contiguous writes); each plane is
                # stored as soon as its ACT finishes.
                ot = o_pool.tile([TILE_P, FREE_OUT], F32, tag="ot")
                ts3 = t[:].rearrange("p (j c) -> p c j", c=C)
                o3 = ot[:].rearrange("p (c j) -> p c j", c=C)
                for c in range(C):
                    nc.scalar.activation(
                        o3[:, c], ts3[:, c],
                        mybir.ActivationFunctionType.Identity,
                        bias=sbt[:, 4 + c:5 + c],
                        scale=sbt[:, c:c + 1],
                    )
                    nc.sync.dma_start(out[b, c, i0:i0 + TILE_P, :], o3[:, c])

    return nc


def _split_multi_waits(nc):
    """walrus codegen accepts at most one semaphore wait per instruction;
    this Tile version can leave several in sync_info.on_wait. Move the
    extras onto same-engine InstNoOp carriers inserted just before."""
    n_split = 0
    for bb in nc.main_func.blocks:
        new_insts = []
        for ins in bb.instructions:
            si = ins.sync_info
            if si is not None and si.on_wait is not None and len(si.on_wait) > 1:
                waits = list(si.on_wait)
                for w in waits[:-1]:
                    nop = mybir.InstNoOp(
                        name=nc.get_next_instruction_name(),
                        engine=ins.engine,
                        ins=[],
                        outs=[],
                        sync_info=mybir.SyncInfo(on_wait=[w], on_update=[]),
                    )
                    new_insts.append(nop)
                ins.sync_info = mybir.SyncInfo(
                    on_wait=[waits[-1]], on_update=list(si.on_update or [])
                )
                n_split += 1
            new_insts.append(ins)
        bb.instructions[:] = new_insts
    return n_split


def _get_nc():
    if "nc" not in _BUILT_CACHE:
        nc = _build_nc()
        _split_multi_waits(nc)
        _BUILT_CACHE["nc"] = nc
    return _BUILT_CACHE["nc"]


def run(images, mean, std, trace=False, **spmd_kwargs):
    images = np.ascontiguousarray(np.asarray(images, dtype=np.float32))
    mean = np.asarray(mean, dtype=np.float32).reshape(-1)
    std = np.asarray(std, dtype=np.float32).reshape(-1)
    assert images.shape == (B_FULL, H_IN, W_IN, C), images.shape

    # ACT input is 8x the resized value (3+1 weights on v = 2x vertical sum)
    scale = 0.125 / (255.0 * std.astype(np.float64))
    bias = -(mean.astype(np.float64) / std.astype(np.float64))
    sbarr = np.zeros((TILE_P, 8), dtype=np.float32)
    sbarr[:, 0:3] = scale.astype(np.float32)
    sbarr[:, 4:7] = bias.astype(np.float32)

    nc = _get_nc()
    in_maps = [
        {"images": np.ascontiguousarray(images[i * PER_B:(i + 1) * PER_B]), "sb": sbarr}
        for i in range(N_CORES)
    ]
    res = run_bass_kernel_spmd(nc, in_maps, list(range(N_CORES)), trace=trace, **spmd_kwargs)
    outs = np.concatenate([r["out"] for r in res.results], axis=0)
    return outs, res


def kernel(**inputs):
    outs, _ = run(inputs["images"], inputs["mean"], inputs["std"], trace=False)
    return outs
